# revision 19
# baseline (speedup 1.0000x reference)
import sys
sys.path.insert(0, "/opt/trn_rl_repo")
import math
import os
import numpy as np
import ml_dtypes

import concourse.bacc as bacc
import concourse.bass as bass
import concourse.mybir as mybir
import concourse.tile as tile
from concourse.bass_utils import run_bass_kernel_spmd
from concourse.masks import make_identity

bf16 = ml_dtypes.bfloat16
fp8 = ml_dtypes.float8_e4m3
F32 = mybir.dt.float32
BF16 = mybir.dt.bfloat16
FP8 = mybir.dt.float8e4
I16 = mybir.dt.int16

N = 50000
E = 800000
IN = 512
H1, D1 = 4, 64
HD1 = 256
H2, D2 = 1, 64
NCORES = 8
NSH = N // NCORES          # 6250 nodes per core
P = 128
NBLK = math.ceil(NSH / P)  # 49
HALF = 3200                # local-offset split: A = off<3200 (blocks 0-24), B = rest
NBLK_A = HALF // P         # 25
HB = NSH - HALF            # 3050
SP = False
RW1 = 384                  # T1 row: 256 feat bf16 (d,h-order) | 4 el f32 | pad  (768B)
P1 = 384
RW2 = 128                  # T2 row: 64 feat bf16 | 1 el f32 | pad  (256B)
P2 = 128
GB1 = int(os.environ.get("K_GB1", "2"))   # dst-blocks per L1 gather instruction
GB2 = int(os.environ.get("K_GB2", "4"))   # dst-blocks per L2 gather instruction


def _wrap16(idx):
    """[n] ints -> [128, n//16] int16 gather-index layout (16-partition wrap, x8 replicated)."""
    n = len(idx)
    assert n % 16 == 0
    a = np.asarray(idx, dtype=np.int16).reshape(n // 16, 16).T
    return np.tile(a, (8, 1))


def _prep_edges(src, dst):
    """Host-side edge sharding/ordering.

    Per device: edges grouped by dst-block (49 blocks of 128 dst rows), within a
    block split into table A (src local-offset < HALF) and table B, each sorted by
    gather row index for locality. Returns per-device index arrays, dst-offset
    tables, and one-hot S' (dst-partition orientation, fp8) for the er-select
    matmuls.
    """
    src = np.asarray(src).astype(np.int64)
    dst = np.asarray(dst).astype(np.int64)

    dev_lists = []  # [d][b] -> (idxA, offA, idxB, offB)
    for d in range(NCORES):
        m = (dst >= NSH * d) & (dst < NSH * (d + 1))
        s_d = src[m]
        t_d = dst[m] - NSH * d
        o = np.argsort(t_d, kind="stable")
        s_d, t_d = s_d[o], t_d[o]
        blk = t_d // P
        core = s_d // NSH
        off = s_d % NSH
        isA = off < HALF
        idxA_all = core * HALF + off
        idxB_all = core * HB + (off - HALF)
        blocks = []
        for b in range(NBLK):
            mb = blk == b
            ma = mb & isA
            mb_ = mb & ~isA
            ia, oa = idxA_all[ma], t_d[ma] - P * b
            ib, ob = idxB_all[mb_], t_d[mb_] - P * b
            # sort by gather address for DMA locality
            sa = np.argsort(ia, kind="stable")
            sb = np.argsort(ib, kind="stable")
            blocks.append((ia[sa], oa[sa], ib[sb], ob[sb]))
        dev_lists.append(blocks)

    nA = np.zeros(NBLK, dtype=np.int64)
    nB = np.zeros(NBLK, dtype=np.int64)
    for b in range(NBLK):
        for d in range(NCORES):
            ia, _, ib, _ = dev_lists[d][b]
            nA[b] = max(nA[b], (len(ia) + P - 1) // P)
            nB[b] = max(nB[b], (len(ib) + P - 1) // P)
        if nA[b] + nB[b] == 0:
            nA[b] = 1
    T = nA + nB
    NT = int(T.sum())

    idx_lo, idx_hi, doff, sprime = [], [], [], []
    for d in range(NCORES):
        lo_cols, hi_cols, do_cols = [], [], []
        sp_d = np.zeros((128, NT * 128), dtype=np.float32)
        ct = 0
        for b in range(NBLK):
            ia, oa, ib, ob = dev_lists[d][b]
            npadA = nA[b] * P - len(ia)
            npadB = nB[b] * P - len(ib)
            a_i = np.concatenate([ia, np.zeros(npadA, np.int64)])
            a_o = np.concatenate([oa, np.full(npadA, -1.0)])
            b_i = np.concatenate([ib, np.zeros(npadB, np.int64)])
            b_o = np.concatenate([ob, np.full(npadB, -1.0)])
            if nA[b]:
                lo_cols.append(_wrap16(a_i))
            if nB[b]:
                hi_cols.append(_wrap16(b_i))
            do = np.concatenate([a_o, b_o]).astype(np.float32)
            do_cols.append(do.reshape(T[b], P).T)
            valid = do >= 0
            cols = 128 * ct + np.arange(T[b] * 128)
            sp_d[do[valid].astype(np.int64), cols[valid]] = 1.0
            ct += T[b]
        idx_lo.append(np.hstack(lo_cols).astype(np.int16))
        idx_hi.append(np.hstack(hi_cols).astype(np.int16))
        doff.append(np.hstack(do_cols).astype(np.float32))
        sprime.append(sp_d.astype(fp8))
    return nA, nB, NT, idx_lo, idx_hi, doff, sprime


def _perm_dh():
    """Permutation p with permuted_feat[4*d+h] = feat[64*h+d]."""
    p = np.zeros(HD1, dtype=np.int64)
    for h in range(H1):
        for d_ in range(D1):
            p[4 * d_ + h] = 64 * h + d_
    return p


def _ap3(t, off, d1, d2):
    """AP keeping t's partition dim, with free dims d1=[stride,count], d2 and element offset off."""
    return bass.AP(t.tensor, t.offset + off, [t.ap[0], list(d1), list(d2)])


def _build(nA, nB, NT, CL, CH, has_b1, has_b2):
    NQ = int(os.environ.get("K_QUEUES", "4"))
    SKIP_P3 = bool(int(os.environ.get("K_SKIP_P3", "0")))
    SKIP_P6 = bool(int(os.environ.get("K_SKIP_P6", "0")))
    SKIP_AG = bool(int(os.environ.get("K_SKIP_AG", "0")))
    nc = bacc.Bacc("TRN2", target_bir_lowering=False, debug=False, num_devices=NCORES,
                   num_swdge_queues=NQ)
    qctr = [0]
    def nextq():
        q = qctr[0] % NQ
        qctr[0] += 1
        return q

    EXP = mybir.ActivationFunctionType.Exp
    RELU = mybir.ActivationFunctionType.Relu
    COPY = mybir.ActivationFunctionType.Copy

    xT = nc.dram_tensor("xT", [IN, NSH], F32, kind="ExternalInput")
    w1e = nc.dram_tensor("w1e", [IN, 264], F32, kind="ExternalInput")
    w2e = nc.dram_tensor("w2e", [HD1, 66], F32, kind="ExternalInput")
    ilo = nc.dram_tensor("ilo", [128, max(CL, 1)], I16, kind="ExternalInput")
    ihi = nc.dram_tensor("ihi", [128, max(CH, 1)], I16, kind="ExternalInput")
    idoff = nc.dram_tensor("idoff", [128, NT], F32, kind="ExternalInput")
    spT = nc.dram_tensor("spT", [128, NT * 128], FP8, kind="ExternalInput")
    if has_b1:
        b1r = nc.dram_tensor("b1r", [128, HD1], F32, kind="ExternalInput")
    if has_b2:
        b2r = nc.dram_tensor("b2r", [128, D2], F32, kind="ExternalInput")
    out_t = nc.dram_tensor("out", [NSH, D2], F32, kind="ExternalOutput")

    iota_np = np.tile(np.arange(128, dtype=bf16)[None, :], (128, 1))
    iota_d = nc.inline_tensor(iota_np, name="iota_c")

    ps_last = NSH - P * (NBLK - 1)  # rows in last block (106)
    TBMAX = int(max(nA[b] + nB[b] for b in range(NBLK)))
    # cumulative tile starts per block
    ctv = np.concatenate([[0], np.cumsum(nA + nB)]).astype(np.int64)
    clov = np.concatenate([[0], np.cumsum(nA)]).astype(np.int64)
    chiv = np.concatenate([[0], np.cumsum(nB)]).astype(np.int64)

    with tile.TileContext(nc) as tc:
        with (
            tc.tile_pool(name="const", bufs=1) as cpool,
            tc.tile_pool(name="dram", bufs=1, space="DRAM") as dram,
        ):
            iota_t = cpool.tile([128, 128], BF16)
            nc.sync.dma_start(out=iota_t[:], in_=iota_d[:, :])
            ident = cpool.tile([128, 128], BF16)
            make_identity(nc, ident[:])

            ilo_t = cpool.tile([128, max(CL, 1)], I16)
            ihi_t = cpool.tile([128, max(CH, 1)], I16)
            doff_t = cpool.tile([128, NT], F32)
            nc.sync.dma_start(out=ilo_t[:], in_=ilo[:, :])
            nc.sync.dma_start(out=ihi_t[:], in_=ihi[:, :])
            nc.sync.dma_start(out=doff_t[:], in_=idoff[:, :])
            if has_b1:
                b1_t = cpool.tile([128, HD1], F32)
                nc.sync.dma_start(out=b1_t[:], in_=b1r[:, :])
            if has_b2:
                b2_t = cpool.tile([128, D2], F32)
                nc.sync.dma_start(out=b2_t[:], in_=b2r[:, :])

            er_sb = cpool.tile([128, 4 * NBLK], BF16, name="er_sb")
            ere2 = cpool.tile([128, NT], BF16, name="ere2")

            # persistent hT (transposed L1 output, input to dense L2)
            hT = []
            for k in range(2):
                hT_k = cpool.tile([128, NBLK * P], BF16, tag=f"hT{k}", name=f"hT{k}")
                hT.append(hT_k)

            T1_localA = dram.tile([HALF, P1], BF16)
            T1_localB = dram.tile([HB, P1], BF16)
            T1_fullA = dram.tile([NCORES * HALF, P1], BF16, addr_space="Shared")
            T1_fullB = dram.tile([NCORES * HB, P1], BF16, addr_space="Shared")
            T2_localA = dram.tile([HALF, P2], BF16)
            T2_localB = dram.tile([HB, P2], BF16)
            T2_fullA = dram.tile([NCORES * HALF, P2], BF16, addr_space="Shared")
            T2_fullB = dram.tile([NCORES * HB, P2], BF16, addr_space="Shared")

            # ---------------- phase 1: dense L1 -> T1_local, er1 -> er_sb ----
            with (
                tc.tile_pool(name="dsb", bufs=1) as dsb,
                tc.tile_pool(name="dps", bufs=3, space="PSUM") as dps,
                tc.tile_pool(name="combop", bufs=3) as combop,
            ):
                rhsW1 = []
                for k in range(4):
                    rhsW1_k = dsb.tile([128, 264], BF16, tag=f"rhsW1{k}", name=f"rhsW1{k}")
                    rhsW1.append(rhsW1_k)
                    nc.gpsimd.dma_start(out=rhsW1_k[:], in_=w1e[128 * k:128 * (k + 1), :])
                xT_t = []
                for k in range(4):
                    xT_k = dsb.tile([128, NSH], BF16, tag=f"xT{k}", name=f"xT{k}")
                    xT_t.append(xT_k)
                    nc.gpsimd.dma_start(out=xT_k[:], in_=xT[128 * k:128 * (k + 1), :])

                for nb in range(NBLK):
                    pb = P if nb < NBLK - 1 else ps_last
                    ps1 = dps.tile([128, 264], F32, tag="ps1")
                    for k in range(4):
                        nc.tensor.matmul(
                            out=ps1[:pb, :], lhsT=xT_t[k][:, P * nb:P * nb + pb],
                            rhs=rhsW1[k][:], start=(k == 0), stop=(k == 3))
                    combo = combop.tile([128, P1], BF16, tag="combo1")
                    nc.vector.tensor_copy(combo[:pb, 0:256], ps1[:pb, 0:256])
                    nc.vector.tensor_copy(
                        combo[:pb, 256:264].bitcast(F32), ps1[:pb, 256:260])
                    nc.vector.tensor_copy(er_sb[:pb, 4 * nb:4 * nb + 4], ps1[:pb, 260:264])
                    if nb < NBLK_A:
                        nc.sync.dma_start(
                            out=T1_localA[P * nb:P * nb + pb, :], in_=combo[:pb, :])
                    else:
                        r0 = P * nb - HALF
                        nc.sync.dma_start(
                            out=T1_localB[r0:r0 + pb, :], in_=combo[:pb, :])
                    if nb == NBLK_A - 1 and not SKIP_AG:
                        nc.gpsimd.collective_compute(
                            "AllGather", mybir.AluOpType.bypass,
                            replica_groups=[list(range(NCORES))],
                            ins=[T1_localA[:, :]], outs=[T1_fullA[:, :]])
                if not SKIP_AG:
                    nc.gpsimd.collective_compute(
                        "AllGather", mybir.AluOpType.bypass,
                        replica_groups=[list(range(NCORES))],
                        ins=[T1_localB[:, :]], outs=[T1_fullB[:, :]])

            # ---------------- phase 3: L1 edge aggregation + fused dense L2 + er2 ----
            with (
                tc.tile_pool(name="gbuf", bufs=2) as gbuf,
                tc.tile_pool(name="esb", bufs=2) as esb,
                tc.tile_pool(name="spp", bufs=2) as spp,
                tc.tile_pool(name="eps", bufs=2, space="PSUM") as eps,
                tc.tile_pool(name="erps", bufs=2, space="PSUM") as erps,
                tc.tile_pool(name="tps", bufs=2, space="PSUM") as tps,
                tc.tile_pool(name="d2sb", bufs=1) as d2sb,
                tc.tile_pool(name="combop2", bufs=3) as combop2,
            ):
                rhsW2 = []
                for k in range(2):
                    rhsW2_k = d2sb.tile([128, 66], BF16, tag=f"rhsW2{k}", name=f"rhsW2{k}")
                    rhsW2.append(rhsW2_k)
                    nc.gpsimd.dma_start(out=rhsW2_k[:], in_=w2e[128 * k:128 * (k + 1), :])

                for s0 in range(0, 0 if SKIP_P3 else NBLK, GB1):
                    sblocks = list(range(s0, min(s0 + GB1, NBLK)))
                    sa = int(sum(nA[b] for b in sblocks))
                    sb_ = int(sum(nB[b] for b in sblocks))
                    bufA = gbuf.tile([128, max(sa, 1) * RW1], BF16, tag="bufA")
                    bufB = gbuf.tile([128, max(sb_, 1) * RW1], BF16, tag="bufB")
                    if sa:
                        nc.gpsimd.dma_gather(
                            out_ap=bufA[:, 0:sa * RW1].rearrange("p (t e) -> p t e", e=RW1),
                            in_ap=T1_fullA[:, :],
                            idxs_ap=ilo_t[:, clov[s0] * 8:(clov[s0] + sa) * 8],
                            num_idxs=sa * P, num_idxs_reg=sa * P, elem_size=RW1,
                            queue_num=nextq(), single_packet=SP)
                    if sb_:
                        nc.gpsimd.dma_gather(
                            out_ap=bufB[:, 0:sb_ * RW1].rearrange("p (t e) -> p t e", e=RW1),
                            in_ap=T1_fullB[:, :],
                            idxs_ap=ihi_t[:, chiv[s0] * 8:(chiv[s0] + sb_) * 8],
                            num_idxs=sb_ * P, num_idxs_reg=sb_ * P, elem_size=RW1,
                            queue_num=nextq(), single_packet=SP)

                    for b in sblocks:
                        a, bb = int(nA[b]), int(nB[b])
                        t_b = a + bb
                        ct = int(ctv[b])
                        pb = P if b < NBLK - 1 else ps_last
                        aoff = int(clov[b] - clov[s0])   # tile offset of this block in bufA
                        boff = int(chiv[b] - chiv[s0])

                        # S' load (dst-partition one-hot, fp8) for er selects
                        spb = spp.tile([128, TBMAX * 128], FP8, tag="spb")
                        nc.sync.dma_start(
                            out=spb[:, 0:t_b * 128],
                            in_=spT[:, 128 * ct:128 * (ct + t_b)])

                        # er1 select: er_ps[e, 4t:4t+4] = er1[dst(e)]
                        er_ps = erps.tile([128, 4 * TBMAX + 66 + TBMAX], F32, tag="er_ps")
                        for t in range(t_b):
                            nc.tensor.matmul(
                                out=er_ps[:, 4 * t:4 * t + 4],
                                lhsT=spb[:, 128 * t:128 * (t + 1)],
                                rhs=er_sb[:, 4 * b:4 * b + 4], start=True, stop=True)

                        # S build (edge-partition one-hot) for scatter
                        S_all = esb.tile([128, TBMAX * 128], BF16, tag="Sall3")
                        for t in range(t_b):
                            nc.vector.tensor_scalar(
                                out=S_all[:, 128 * t:128 * (t + 1)], in0=iota_t[:],
                                scalar1=doff_t[:, ct + t:ct + t + 1], scalar2=None,
                                op0=mybir.AluOpType.is_equal)

                        # z = el_src + er_dst ; ex = exp(leaky_relu(z))
                        z = esb.tile([128, t_b * 4], F32, tag="z")
                        zr = z[:].rearrange("p (t h) -> p t h", h=4)
                        if a:
                            elA = _ap3(bufA[:].bitcast(F32), aoff * 192 + 128, [192, a], [1, 4])
                            nc.vector.tensor_tensor(
                                out=zr[:, 0:a, :], in0=elA,
                                in1=_ap3(er_ps[:], 0, [4, a], [1, 4]),
                                op=mybir.AluOpType.add)
                        if bb:
                            elB = _ap3(bufB[:].bitcast(F32), boff * 192 + 128, [192, bb], [1, 4])
                            nc.vector.tensor_tensor(
                                out=zr[:, a:t_b, :], in0=elB,
                                in1=_ap3(er_ps[:], 4 * a, [4, bb], [1, 4]),
                                op=mybir.AluOpType.add)
                        e1 = esb.tile([128, t_b * 4], F32, tag="e1")
                        e2 = esb.tile([128, t_b * 4], F32, tag="e2")
                        nc.scalar.activation(out=e1[:], in_=z[:], func=EXP)
                        nc.scalar.activation(out=e2[:], in_=z[:], func=EXP, scale=0.2)
                        ex = esb.tile([128, t_b * 4], BF16, tag="ex")
                        nc.vector.tensor_tensor(out=ex[:], in0=e1[:], in1=e2[:], op=mybir.AluOpType.max)

                        # rhs[e, 260t + 4d + h] = buf[e, t, (d,h)] * ex[e, t, h]; cols 256:260 = ex
                        rhs = esb.tile([128, t_b * 260], BF16, tag="rhs")
                        if a:
                            nc.vector.tensor_tensor(
                                out=bass.AP(rhs[:].tensor, rhs[:].offset,
                                            [rhs[:].ap[0], [260, a], [4, 64], [1, 4]]),
                                in0=bass.AP(bufA[:].tensor, bufA[:].offset + aoff * RW1,
                                            [bufA[:].ap[0], [RW1, a], [4, 64], [1, 4]]),
                                in1=bass.AP(ex[:].tensor, ex[:].offset,
                                            [ex[:].ap[0], [4, a], [0, 64], [1, 4]]),
                                op=mybir.AluOpType.mult)
                        if bb:
                            nc.vector.tensor_tensor(
                                out=bass.AP(rhs[:].tensor, rhs[:].offset + 260 * a,
                                            [rhs[:].ap[0], [260, bb], [4, 64], [1, 4]]),
                                in0=bass.AP(bufB[:].tensor, bufB[:].offset + boff * RW1,
                                            [bufB[:].ap[0], [RW1, bb], [4, 64], [1, 4]]),
                                in1=bass.AP(ex[:].tensor, ex[:].offset + 4 * a,
                                            [ex[:].ap[0], [4, bb], [0, 64], [1, 4]]),
                                op=mybir.AluOpType.mult)
                        nc.vector.tensor_copy(
                            rhs[:].rearrange("p (t c) -> p t c", c=260)[:, :, 256:260],
                            ex[:].rearrange("p (t h) -> p t h", h=4))

                        ps_o = eps.tile([128, 260], F32, tag="ps_o")
                        for t in range(t_b):
                            nc.tensor.matmul(
                                out=ps_o[:], lhsT=S_all[:, 128 * t:128 * (t + 1)],
                                rhs=rhs[:, 260 * t:260 * t + 260],
                                start=(t == 0), stop=(t == t_b - 1))

                        # normalize (cols are (d,h)-ordered; denominators at 256:260 by h)
                        splus = esb.tile([128, 4], F32, tag="splus")
                        nc.vector.tensor_scalar(
                            out=splus[:], in0=ps_o[:, 256:260], scalar1=1e-30,
                            scalar2=None, op0=mybir.AluOpType.add)
                        r = esb.tile([128, 4], F32, tag="r")
                        nc.vector.reciprocal(r[:], splus[:])
                        xn = esb.tile([128, 256], F32, tag="xn")
                        r_b = bass.AP(r[:].tensor, r[:].offset, [r[:].ap[0], [0, 64], [1, 4]])
                        nc.vector.tensor_tensor(
                            out=xn[:].rearrange("p (d h) -> p d h", h=4),
                            in0=ps_o[:, 0:256].rearrange("p (d h) -> p d h", h=4),
                            in1=r_b, op=mybir.AluOpType.mult)
                        if has_b1:
                            nc.vector.tensor_tensor(out=xn[:], in0=xn[:], in1=b1_t[:], op=mybir.AluOpType.add)
                        # elu(x) = exp(min(x,0)) + (max(x,0) - 1)
                        t1 = esb.tile([128, 256], F32, tag="t1")
                        nc.scalar.activation(out=t1[:], in_=xn[:], func=RELU, scale=-1.0)
                        u = esb.tile([128, 256], F32, tag="u")
                        nc.scalar.activation(out=u[:], in_=t1[:], func=EXP, scale=-1.0)
                        v = esb.tile([128, 256], F32, tag="v")
                        nc.vector.tensor_scalar(
                            out=v[:], in0=xn[:], scalar1=0.0, scalar2=-1.0,
                            op0=mybir.AluOpType.max, op1=mybir.AluOpType.add)
                        hb = esb.tile([128, 256], BF16, tag="hb")
                        nc.vector.tensor_tensor(out=hb[:], in0=u[:], in1=v[:], op=mybir.AluOpType.add)
                        pst = tps.tile([128, 256], BF16, tag="pst")
                        for k2 in range(2):
                            nc.tensor.transpose(out=pst[:, 128 * k2:128 * (k2 + 1)],
                                                in_=hb[:, 128 * k2:128 * (k2 + 1)], identity=ident[:])
                        for k2 in range(2):
                            nc.vector.tensor_copy(hT[k2][:, P * b:P * (b + 1)],
                                                  pst[:, 128 * k2:128 * (k2 + 1)])

                        # dense L2 (fused): ps2 cols 0:64 feat2, 64 el2, 65 er2
                        ps2 = er_ps[:, 4 * TBMAX:4 * TBMAX + 66]
                        for k in range(2):
                            nc.tensor.matmul(
                                out=ps2[:pb, :], lhsT=hT[k][:, P * b:P * b + pb],
                                rhs=rhsW2[k][:], start=(k == 0), stop=(k == 1))
                        er2b = esb.tile([128, 1], BF16, tag="er2b")
                        nc.vector.tensor_copy(er2b[:pb, :], ps2[:pb, 65:66])
                        # er2 select per tile (reuses S'); padded dst rows give garbage er2 — harmless
                        er2_ps = er_ps[:, 4 * TBMAX + 66:4 * TBMAX + 66 + TBMAX]
                        for t in range(t_b):
                            nc.tensor.matmul(
                                out=er2_ps[:, t:t + 1],
                                lhsT=spb[:, 128 * t:128 * (t + 1)],
                                rhs=er2b[:], start=True, stop=True)
                        nc.vector.tensor_copy(ere2[:, ct:ct + t_b], er2_ps[:, 0:t_b])

                        combo2 = combop2.tile([128, P2], BF16, tag="combo2")
                        nc.vector.tensor_copy(combo2[:pb, 0:64], ps2[:pb, 0:64])
                        nc.vector.tensor_copy(combo2[:pb, 64:66].bitcast(F32), ps2[:pb, 64:65])
                        if b < NBLK_A:
                            nc.sync.dma_start(out=T2_localA[P * b:P * b + pb, :], in_=combo2[:pb, :])
                        else:
                            r0 = P * b - HALF
                            nc.sync.dma_start(out=T2_localB[r0:r0 + pb, :], in_=combo2[:pb, :])
                        if b == NBLK_A - 1 and not SKIP_AG:
                            nc.gpsimd.collective_compute(
                                "AllGather", mybir.AluOpType.bypass,
                                replica_groups=[list(range(NCORES))],
                                ins=[T2_localA[:, :]], outs=[T2_fullA[:, :]])
                if not SKIP_AG and not SKIP_P3:
                    nc.gpsimd.collective_compute(
                        "AllGather", mybir.AluOpType.bypass,
                        replica_groups=[list(range(NCORES))],
                        ins=[T2_localB[:, :]], outs=[T2_fullB[:, :]])

            # ---------------- phase 6: L2 edge aggregation ----------------
            with (
                tc.tile_pool(name="g2buf", bufs=2) as g2buf,
                tc.tile_pool(name="e2sb", bufs=2) as e2sb,
                tc.tile_pool(name="e2ps", bufs=3, space="PSUM") as e2ps,
            ):
                for s0 in range(0, 0 if SKIP_P6 else NBLK, GB2):
                    sblocks = list(range(s0, min(s0 + GB2, NBLK)))
                    sa = int(sum(nA[b] for b in sblocks))
                    sb_ = int(sum(nB[b] for b in sblocks))
                    bufA = g2buf.tile([128, max(sa, 1) * RW2], BF16, tag="bufA2")
                    bufB = g2buf.tile([128, max(sb_, 1) * RW2], BF16, tag="bufB2")
                    if sa:
                        nc.gpsimd.dma_gather(
                            out_ap=bufA[:, 0:sa * RW2].rearrange("p (t e) -> p t e", e=RW2),
                            in_ap=T2_fullA[:, :],
                            idxs_ap=ilo_t[:, clov[s0] * 8:(clov[s0] + sa) * 8],
                            num_idxs=sa * P, num_idxs_reg=sa * P, elem_size=RW2,
                            queue_num=nextq(), single_packet=SP)
                    if sb_:
                        nc.gpsimd.dma_gather(
                            out_ap=bufB[:, 0:sb_ * RW2].rearrange("p (t e) -> p t e", e=RW2),
                            in_ap=T2_fullB[:, :],
                            idxs_ap=ihi_t[:, chiv[s0] * 8:(chiv[s0] + sb_) * 8],
                            num_idxs=sb_ * P, num_idxs_reg=sb_ * P, elem_size=RW2,
                            queue_num=nextq(), single_packet=SP)

                    for b in sblocks:
                        a, bb = int(nA[b]), int(nB[b])
                        t_b = a + bb
                        ct = int(ctv[b])
                        pb = P if b < NBLK - 1 else ps_last
                        aoff = int(clov[b] - clov[s0])
                        boff = int(chiv[b] - chiv[s0])

                        S_all = e2sb.tile([128, TBMAX * 128], BF16, tag="Sall6")
                        for t in range(t_b):
                            nc.vector.tensor_scalar(
                                out=S_all[:, 128 * t:128 * (t + 1)], in0=iota_t[:],
                                scalar1=doff_t[:, ct + t:ct + t + 1], scalar2=None,
                                op0=mybir.AluOpType.is_equal)

                        z = e2sb.tile([128, t_b], F32, tag="z2")
                        if a:
                            elA = _ap3(bufA[:].bitcast(F32), aoff * 64 + 32, [64, a], [1, 1])
                            nc.vector.tensor_tensor(
                                out=z[:, 0:a].rearrange("p (t h) -> p t h", h=1),
                                in0=elA,
                                in1=_ap3(ere2[:], ct, [1, a], [1, 1]),
                                op=mybir.AluOpType.add)
                        if bb:
                            elB = _ap3(bufB[:].bitcast(F32), boff * 64 + 32, [64, bb], [1, 1])
                            nc.vector.tensor_tensor(
                                out=z[:, a:t_b].rearrange("p (t h) -> p t h", h=1),
                                in0=elB,
                                in1=_ap3(ere2[:], ct + a, [1, bb], [1, 1]),
                                op=mybir.AluOpType.add)
                        e1 = e2sb.tile([128, t_b], F32, tag="e21")
                        e2 = e2sb.tile([128, t_b], F32, tag="e22")
                        nc.scalar.activation(out=e1[:], in_=z[:], func=EXP)
                        nc.scalar.activation(out=e2[:], in_=z[:], func=EXP, scale=0.2)
                        m = e2sb.tile([128, t_b], F32, tag="m2")
                        nc.vector.tensor_tensor(out=m[:], in0=e1[:], in1=e2[:], op=mybir.AluOpType.max)
                        # ex64[e, t, d] = ex[e, t] broadcast over d
                        ex64 = e2sb.tile([128, t_b * 64], BF16, tag="ex64")
                        nc.scalar.activation(
                            out=ex64[:].rearrange("p (t d) -> p t d", d=64),
                            in_=bass.AP(m[:].tensor, m[:].offset, [m[:].ap[0], [1, t_b], [0, 64]]),
                            func=COPY)
                        rhs = e2sb.tile([128, t_b * 65], BF16, tag="rhs2")
                        if a:
                            nc.vector.tensor_tensor(
                                out=_ap3(rhs[:], 0, [65, a], [1, 64]),
                                in0=bass.AP(bufA[:].tensor, bufA[:].offset + aoff * RW2,
                                            [bufA[:].ap[0], [RW2, a], [1, 64]]),
                                in1=_ap3(ex64[:], 0, [64, a], [1, 64]),
                                op=mybir.AluOpType.mult)
                        if bb:
                            nc.vector.tensor_tensor(
                                out=_ap3(rhs[:], 65 * a, [65, bb], [1, 64]),
                                in0=bass.AP(bufB[:].tensor, bufB[:].offset + boff * RW2,
                                            [bufB[:].ap[0], [RW2, bb], [1, 64]]),
                                in1=_ap3(ex64[:], 64 * a, [64, bb], [1, 64]),
                                op=mybir.AluOpType.mult)
                        nc.vector.tensor_copy(
                            rhs[:].rearrange("p (t c) -> p t c", c=65)[:, :, 64:65],
                            ex64[:].rearrange("p (t d) -> p t d", d=64)[:, :, 0:1])

                        ps_o = e2ps.tile([128, 65], F32, tag="ps_o2")
                        for t in range(t_b):
                            nc.tensor.matmul(
                                out=ps_o[:], lhsT=S_all[:, 128 * t:128 * (t + 1)],
                                rhs=rhs[:, 65 * t:65 * t + 65],
                                start=(t == 0), stop=(t == t_b - 1))

                        splus = e2sb.tile([128, 1], F32, tag="splus2")
                        nc.vector.tensor_scalar(
                            out=splus[:], in0=ps_o[:, 64:65], scalar1=1e-30,
                            scalar2=None, op0=mybir.AluOpType.add)
                        r = e2sb.tile([128, 1], F32, tag="r2")
                        nc.vector.reciprocal(r[:], splus[:])
                        outf = e2sb.tile([128, 64], F32, tag="outf")
                        nc.vector.tensor_scalar(
                            out=outf[:], in0=ps_o[:, 0:64], scalar1=r[:, 0:1],
                            scalar2=None, op0=mybir.AluOpType.mult)
                        if has_b2:
                            nc.vector.tensor_tensor(out=outf[:], in0=outf[:], in1=b2_t[:], op=mybir.AluOpType.add)
                        nc.sync.dma_start(out=out_t[P * b:P * b + pb, :], in_=outf[:pb, :])

    nc.compile()
    return nc


def _make_in_maps(np_inputs, nA, nB, NT, idx_lo, idx_hi, doff, sprime,
                  has_b1, has_b2):
    x = np.asarray(np_inputs["x"], dtype=np.float32)
    W1 = np.asarray(np_inputs["W1"], dtype=np.float32)
    al1 = np.asarray(np_inputs["al1"], dtype=np.float32)
    ar1 = np.asarray(np_inputs["ar1"], dtype=np.float32)
    b1 = np.asarray(np_inputs["b1"], dtype=np.float32)
    W2 = np.asarray(np_inputs["W2"], dtype=np.float32)
    al2 = np.asarray(np_inputs["al2"], dtype=np.float32)
    ar2 = np.asarray(np_inputs["ar2"], dtype=np.float32)
    b2 = np.asarray(np_inputs["b2"], dtype=np.float32)

    p = _perm_dh()
    w1ext = np.zeros((IN, 264), np.float32)
    w1ext[:, 0:256] = W1[:, p]
    for h in range(H1):
        w1ext[:, 256 + h] = W1[:, 64 * h:64 * h + 64] @ al1[h]
        w1ext[:, 260 + h] = W1[:, 64 * h:64 * h + 64] @ ar1[h]
    # L2: el2/er2 contract over the (permuted) 256-dim input of layer 2
    # reference: feat2 = h @ W2 [N, 64]; el2 = (feat2 * al2).sum(-1) = h @ (W2 @ al2)
    w2ext = np.zeros((HD1, 66), np.float32)
    w2ext[:, 0:64] = W2[p, :]
    w2ext[:, 64] = (W2 @ al2[0])[p]
    w2ext[:, 65] = (W2 @ ar2[0])[p]

    xt_np = np.ascontiguousarray(x.T)
    in_maps = []
    for d in range(NCORES):
        m = {
            "xT": np.ascontiguousarray(xt_np[:, NSH * d:NSH * (d + 1)]),
            "w1e": w1ext, "w2e": w2ext,
            "ilo": np.ascontiguousarray(idx_lo[d]),
            "ihi": np.ascontiguousarray(idx_hi[d]),
            "idoff": np.ascontiguousarray(doff[d]),
            "spT": np.ascontiguousarray(sprime[d]),
        }
        if has_b1:
            m["b1r"] = np.tile(b1.reshape(1, HD1)[:, p], (128, 1)).astype(np.float32)
        if has_b2:
            m["b2r"] = np.tile(b2.reshape(1, D2), (128, 1)).astype(np.float32)
        in_maps.append(m)
    return in_maps


def kernel(x, src, dst, W1, al1, ar1, b1, W2, al2, ar2, b2):
    np_inputs = {"x": x, "src": src, "dst": dst, "W1": W1, "al1": al1, "ar1": ar1,
                 "b1": b1, "W2": W2, "al2": al2, "ar2": ar2, "b2": b2}
    nA, nB, NT, idx_lo, idx_hi, doff, sprime = _prep_edges(src, dst)
    CL, CH = idx_lo[0].shape[1], idx_hi[0].shape[1]
    has_b1 = bool(np.any(np.asarray(b1)))
    has_b2 = bool(np.any(np.asarray(b2)))

    nc = _build(nA, nB, NT, CL, CH, has_b1, has_b2)
    in_maps = _make_in_maps(np_inputs, nA, nB, NT, idx_lo, idx_hi, doff, sprime,
                            has_b1, has_b2)

    trace = bool(int(os.environ.get("K_TRACE", "0")))
    res = run_bass_kernel_spmd(
        nc, in_maps, core_ids=list(range(NCORES)), trace=trace,
        trace_cores=list(range(NCORES)) if trace else None, stitch_traces=trace)
    out = np.concatenate([res.results[d]["out"] for d in range(NCORES)], axis=0)
    return out


# revision 27
# speedup vs baseline: 1.6439x; 1.6439x over previous
import sys
sys.path.insert(0, "/opt/trn_rl_repo")
import math
import os
import numpy as np
import ml_dtypes

import concourse.bacc as bacc
import concourse.bass as bass
import concourse.mybir as mybir
import concourse.tile as tile
from concourse.bass_utils import run_bass_kernel_spmd
from concourse.masks import make_identity

bf16 = ml_dtypes.bfloat16
fp8 = ml_dtypes.float8_e4m3
F32 = mybir.dt.float32
BF16 = mybir.dt.bfloat16
FP8 = mybir.dt.float8e4
I16 = mybir.dt.int16

N = 50000
E = 800000
IN = 512
H1, D1 = 4, 64
HD1 = 256
H2, D2 = 1, 64
NCORES = 8
NSH = N // NCORES          # 6250 nodes per core
P = 128
NBLK = math.ceil(NSH / P)  # 49
HALF = 3200                # local-offset split: A = off<3200 (blocks 0-24), B = rest
NBLK_A = HALF // P         # 25
HB = NSH - HALF            # 3050
SP = bool(int(os.environ.get("K_SP", "0")))
RW1 = int(os.environ.get("K_RW1", "384"))  # T1 row elems (bf16)
P1 = RW1
RW2 = 128                  # T2 row: 64 feat bf16 | 1 el f32 | pad  (256B)
P2 = 128
GB1 = int(os.environ.get("K_GB1", "1"))   # dst-blocks per L1 gather instruction
GB2 = int(os.environ.get("K_GB2", "1"))   # dst-blocks per L2 gather instruction


def _wrap16(idx):
    """[n] ints -> [128, n//16] int16 gather-index layout (16-partition wrap, x8 replicated)."""
    n = len(idx)
    assert n % 16 == 0
    a = np.asarray(idx, dtype=np.int16).reshape(n // 16, 16).T
    return np.tile(a, (8, 1))


def _prep_edges(src, dst):
    """Host-side edge sharding/ordering.

    Per device: edges grouped by dst-block (49 blocks of 128 dst rows), within a
    block split into table A (src local-offset < HALF) and table B, each sorted by
    gather row index for locality. Returns per-device index arrays, dst-offset
    tables, and one-hot S' (dst-partition orientation, fp8) for the er-select
    matmuls.
    """
    src = np.asarray(src).astype(np.int64)
    dst = np.asarray(dst).astype(np.int64)

    dev_lists = []  # [d][b] -> (idxA, offA, idxB, offB)
    for d in range(NCORES):
        m = (dst >= NSH * d) & (dst < NSH * (d + 1))
        s_d = src[m]
        t_d = dst[m] - NSH * d
        o = np.argsort(t_d, kind="stable")
        s_d, t_d = s_d[o], t_d[o]
        blk = t_d // P
        core = s_d // NSH
        off = s_d % NSH
        isA = off < HALF
        idxA_all = core * HALF + off
        idxB_all = core * HB + (off - HALF)
        blocks = []
        for b in range(NBLK):
            mb = blk == b
            ma = mb & isA
            mb_ = mb & ~isA
            ia, oa = idxA_all[ma], t_d[ma] - P * b
            ib, ob = idxB_all[mb_], t_d[mb_] - P * b
            # sort by gather address for DMA locality
            sa = np.argsort(ia, kind="stable")
            sb = np.argsort(ib, kind="stable")
            blocks.append((ia[sa], oa[sa], ib[sb], ob[sb]))
        dev_lists.append(blocks)

    nA = np.zeros(NBLK, dtype=np.int64)
    nB = np.zeros(NBLK, dtype=np.int64)
    for b in range(NBLK):
        for d in range(NCORES):
            ia, _, ib, _ = dev_lists[d][b]
            nA[b] = max(nA[b], (len(ia) + P - 1) // P)
            nB[b] = max(nB[b], (len(ib) + P - 1) // P)
        if nA[b] + nB[b] == 0:
            nA[b] = 1
    T = nA + nB
    NT = int(T.sum())

    idx_lo, idx_hi, doff, sprime = [], [], [], []
    for d in range(NCORES):
        lo_cols, hi_cols, do_cols = [], [], []
        sp_d = np.zeros((128, NT * 128), dtype=np.float32)
        ct = 0
        for b in range(NBLK):
            ia, oa, ib, ob = dev_lists[d][b]
            npadA = nA[b] * P - len(ia)
            npadB = nB[b] * P - len(ib)
            a_i = np.concatenate([ia, np.zeros(npadA, np.int64)])
            a_o = np.concatenate([oa, np.full(npadA, -1.0)])
            b_i = np.concatenate([ib, np.zeros(npadB, np.int64)])
            b_o = np.concatenate([ob, np.full(npadB, -1.0)])
            if nA[b]:
                lo_cols.append(_wrap16(a_i))
            if nB[b]:
                hi_cols.append(_wrap16(b_i))
            do = np.concatenate([a_o, b_o]).astype(np.float32)
            do_cols.append(do.reshape(T[b], P).T)
            valid = do >= 0
            cols = 128 * ct + np.arange(T[b] * 128)
            sp_d[do[valid].astype(np.int64), cols[valid]] = 1.0
            ct += T[b]
        idx_lo.append(np.hstack(lo_cols).astype(np.int16))
        idx_hi.append(np.hstack(hi_cols).astype(np.int16))
        doff.append(np.hstack(do_cols).astype(np.float32))
        sprime.append(sp_d.astype(fp8))
    return nA, nB, NT, idx_lo, idx_hi, doff, sprime


def _perm_dh():
    """Permutation p with permuted_feat[4*d+h] = feat[64*h+d]."""
    p = np.zeros(HD1, dtype=np.int64)
    for h in range(H1):
        for d_ in range(D1):
            p[4 * d_ + h] = 64 * h + d_
    return p


def _ap3(t, off, d1, d2):
    """AP keeping t's partition dim, with free dims d1=[stride,count], d2 and element offset off."""
    return bass.AP(t.tensor, t.offset + off, [t.ap[0], list(d1), list(d2)])


def _build(nA, nB, NT, CL, CH, has_b1, has_b2):
    NQ = int(os.environ.get("K_QUEUES", "4"))
    SKIP_P3 = bool(int(os.environ.get("K_SKIP_P3", "0")))
    SKIP_P6 = bool(int(os.environ.get("K_SKIP_P6", "0")))
    SKIP_AG = bool(int(os.environ.get("K_SKIP_AG", "0")))
    LATE_AG = bool(int(os.environ.get("K_LATE_AG", "1")))
    P6_STAGE = int(os.environ.get("K_P6_STAGE", "9"))
    P3_STAGE = int(os.environ.get("K_P3_STAGE", "9"))
    SBUILD_TT = bool(int(os.environ.get("K_SBUILD_TT", "0")))
    nc = bacc.Bacc("TRN2", target_bir_lowering=False, debug=False, num_devices=NCORES,
                   num_swdge_queues=NQ)
    qctr = [0]
    def nextq():
        q = qctr[0] % NQ
        qctr[0] += 1
        return q

    EXP = mybir.ActivationFunctionType.Exp
    RELU = mybir.ActivationFunctionType.Relu
    COPY = mybir.ActivationFunctionType.Copy

    xT = nc.dram_tensor("xT", [IN, NSH], F32, kind="ExternalInput")
    w1e = nc.dram_tensor("w1e", [IN, 264], F32, kind="ExternalInput")
    w2e = nc.dram_tensor("w2e", [HD1, 66], F32, kind="ExternalInput")
    ilo = nc.dram_tensor("ilo", [128, max(CL, 1)], I16, kind="ExternalInput")
    ihi = nc.dram_tensor("ihi", [128, max(CH, 1)], I16, kind="ExternalInput")
    idoff = nc.dram_tensor("idoff", [128, NT], F32, kind="ExternalInput")
    spT = nc.dram_tensor("spT", [128, NT * 128], FP8, kind="ExternalInput")
    if has_b1:
        b1r = nc.dram_tensor("b1r", [128, HD1], F32, kind="ExternalInput")
    if has_b2:
        b2r = nc.dram_tensor("b2r", [128, D2], F32, kind="ExternalInput")
    out_t = nc.dram_tensor("out", [NSH, D2], F32, kind="ExternalOutput")

    iota_np = np.tile(np.arange(128, dtype=bf16)[None, :], (128, 1))
    iota_d = nc.inline_tensor(iota_np, name="iota_c")

    ps_last = NSH - P * (NBLK - 1)  # rows in last block (106)
    TBMAX = int(max(nA[b] + nB[b] for b in range(NBLK)))
    # cumulative tile starts per block
    ctv = np.concatenate([[0], np.cumsum(nA + nB)]).astype(np.int64)
    clov = np.concatenate([[0], np.cumsum(nA)]).astype(np.int64)
    chiv = np.concatenate([[0], np.cumsum(nB)]).astype(np.int64)

    with tile.TileContext(nc) as tc:
        with (
            tc.tile_pool(name="const", bufs=1) as cpool,
            tc.tile_pool(name="dram", bufs=1, space="DRAM") as dram,
        ):
            iota_t = cpool.tile([128, 128], BF16)
            nc.sync.dma_start(out=iota_t[:], in_=iota_d[:, :])
            ident = cpool.tile([128, 128], BF16)
            make_identity(nc, ident[:])

            ilo_t = cpool.tile([128, max(CL, 1)], I16)
            ihi_t = cpool.tile([128, max(CH, 1)], I16)
            doff_t = cpool.tile([128, NT], F32)
            nc.sync.dma_start(out=ilo_t[:], in_=ilo[:, :])
            nc.sync.dma_start(out=ihi_t[:], in_=ihi[:, :])
            nc.sync.dma_start(out=doff_t[:], in_=idoff[:, :])
            if has_b1:
                b1_t = cpool.tile([128, HD1], F32)
                nc.sync.dma_start(out=b1_t[:], in_=b1r[:, :])
            if has_b2:
                b2_t = cpool.tile([128, D2], F32)
                nc.sync.dma_start(out=b2_t[:], in_=b2r[:, :])

            er_sb = cpool.tile([128, 4 * NBLK], BF16, name="er_sb")
            ere2 = cpool.tile([128, NT], BF16, name="ere2")
            if SKIP_P3 or P3_STAGE < 9:
                nc.gpsimd.memset(ere2[:], 0)
            if P3_STAGE < 9:
                nc.gpsimd.memset(er_sb[:], 0)
                for k in range(2):
                    nc.gpsimd.memset(hT[k][:], 0) if False else None

            # persistent hT (transposed L1 output, input to dense L2)
            hT = []
            for k in range(2):
                hT_k = cpool.tile([128, NBLK * P], BF16, tag=f"hT{k}", name=f"hT{k}")
                hT.append(hT_k)

            T1_localA = dram.tile([HALF, P1], BF16)
            T1_localB = dram.tile([HB, P1], BF16)
            T1_fullA = dram.tile([NCORES * HALF, P1], BF16, addr_space="Shared")
            T1_fullB = dram.tile([NCORES * HB, P1], BF16, addr_space="Shared")
            T2_localA = dram.tile([HALF, P2], BF16)
            T2_localB = dram.tile([HB, P2], BF16)
            T2_fullA = dram.tile([NCORES * HALF, P2], BF16, addr_space="Shared")
            T2_fullB = dram.tile([NCORES * HB, P2], BF16, addr_space="Shared")

            # ---------------- phase 1: dense L1 -> T1_local, er1 -> er_sb ----
            with (
                tc.tile_pool(name="dsb", bufs=1) as dsb,
                tc.tile_pool(name="dps", bufs=3, space="PSUM") as dps,
                tc.tile_pool(name="combop", bufs=3) as combop,
            ):
                rhsW1 = []
                for k in range(4):
                    rhsW1_k = dsb.tile([128, 264], BF16, tag=f"rhsW1{k}", name=f"rhsW1{k}")
                    rhsW1.append(rhsW1_k)
                    nc.gpsimd.dma_start(out=rhsW1_k[:], in_=w1e[128 * k:128 * (k + 1), :])
                xT_t = []
                for k in range(4):
                    xT_k = dsb.tile([128, NSH], BF16, tag=f"xT{k}", name=f"xT{k}")
                    xT_t.append(xT_k)
                    nc.gpsimd.dma_start(out=xT_k[:], in_=xT[128 * k:128 * (k + 1), :])

                for nb in range(NBLK):
                    pb = P if nb < NBLK - 1 else ps_last
                    ps1 = dps.tile([128, 264], F32, tag="ps1")
                    for k in range(4):
                        nc.tensor.matmul(
                            out=ps1[:pb, :], lhsT=xT_t[k][:, P * nb:P * nb + pb],
                            rhs=rhsW1[k][:], start=(k == 0), stop=(k == 3))
                    combo = combop.tile([128, P1], BF16, tag="combo1")
                    nc.vector.tensor_copy(combo[:pb, 0:256], ps1[:pb, 0:256])
                    if P1 >= 264:
                        nc.vector.tensor_copy(
                            combo[:pb, 256:264].bitcast(F32), ps1[:pb, 256:260])
                    nc.vector.tensor_copy(er_sb[:pb, 4 * nb:4 * nb + 4], ps1[:pb, 260:264])
                    if nb < NBLK_A:
                        nc.sync.dma_start(
                            out=T1_localA[P * nb:P * nb + pb, :], in_=combo[:pb, :])
                    else:
                        r0 = P * nb - HALF
                        nc.sync.dma_start(
                            out=T1_localB[r0:r0 + pb, :], in_=combo[:pb, :])
                    if nb == NBLK_A - 1 and not SKIP_AG and not LATE_AG:
                        nc.gpsimd.collective_compute(
                            "AllGather", mybir.AluOpType.bypass,
                            replica_groups=[list(range(NCORES))],
                            ins=[T1_localA[:, :]], outs=[T1_fullA[:, :]])
                if not SKIP_AG and LATE_AG:
                    nc.gpsimd.collective_compute(
                        "AllGather", mybir.AluOpType.bypass,
                        replica_groups=[list(range(NCORES))],
                        ins=[T1_localA[:, :]], outs=[T1_fullA[:, :]])
                if not SKIP_AG:
                    nc.gpsimd.collective_compute(
                        "AllGather", mybir.AluOpType.bypass,
                        replica_groups=[list(range(NCORES))],
                        ins=[T1_localB[:, :]], outs=[T1_fullB[:, :]])

            # ---------------- phase 3: L1 edge aggregation + fused dense L2 + er2 ----
            with (
                tc.tile_pool(name="gbuf", bufs=3) as gbuf,
                tc.tile_pool(name="esb", bufs=3) as esb,
                tc.tile_pool(name="spp", bufs=3) as spp,
                tc.tile_pool(name="eps", bufs=2, space="PSUM") as eps,
                tc.tile_pool(name="erps", bufs=2, space="PSUM") as erps,
                tc.tile_pool(name="tps", bufs=2, space="PSUM") as tps,
                tc.tile_pool(name="d2sb", bufs=1) as d2sb,
                tc.tile_pool(name="combop2", bufs=3) as combop2,
            ):
                rhsW2 = []
                for k in range(2):
                    rhsW2_k = d2sb.tile([128, 66], BF16, tag=f"rhsW2{k}", name=f"rhsW2{k}")
                    rhsW2.append(rhsW2_k)
                    nc.gpsimd.dma_start(out=rhsW2_k[:], in_=w2e[128 * k:128 * (k + 1), :])

                GCH = int(os.environ.get("K_GCH", "7"))
                for s0 in range(0, 0 if SKIP_P3 else NBLK, GB1):
                    sblocks = list(range(s0, min(s0 + GB1, NBLK)))
                    sa = int(sum(nA[b] for b in sblocks))
                    sb_ = int(sum(nB[b] for b in sblocks))
                    bufA = gbuf.tile([128, max(sa, 1) * RW1], BF16, tag="bufA")
                    bufB = gbuf.tile([128, max(sb_, 1) * RW1], BF16, tag="bufB")
                    for c0 in range(0, sa, GCH):
                        cn = min(GCH, sa - c0)
                        nc.gpsimd.dma_gather(
                            out_ap=bufA[:, RW1 * c0:RW1 * (c0 + cn)].rearrange("p (t e) -> p t e", e=RW1),
                            in_ap=T1_fullA[:, :],
                            idxs_ap=ilo_t[:, (clov[s0] + c0) * 8:(clov[s0] + c0 + cn) * 8],
                            num_idxs=cn * P, num_idxs_reg=cn * P, elem_size=RW1,
                            queue_num=nextq(), single_packet=SP)
                    for c0 in range(0, sb_, GCH):
                        cn = min(GCH, sb_ - c0)
                        nc.gpsimd.dma_gather(
                            out_ap=bufB[:, RW1 * c0:RW1 * (c0 + cn)].rearrange("p (t e) -> p t e", e=RW1),
                            in_ap=T1_fullB[:, :],
                            idxs_ap=ihi_t[:, (chiv[s0] + c0) * 8:(chiv[s0] + c0 + cn) * 8],
                            num_idxs=cn * P, num_idxs_reg=cn * P, elem_size=RW1,
                            queue_num=nextq(), single_packet=SP)

                    for b in sblocks:
                        if P3_STAGE < 1:
                            continue
                        a, bb = int(nA[b]), int(nB[b])
                        t_b = a + bb
                        ct = int(ctv[b])
                        pb = P if b < NBLK - 1 else ps_last
                        aoff = int(clov[b] - clov[s0])   # tile offset of this block in bufA
                        boff = int(chiv[b] - chiv[s0])

                        # S' load (dst-partition one-hot, fp8) for er selects
                        spb = spp.tile([128, TBMAX * 128], FP8, tag="spb")
                        nc.sync.dma_start(
                            out=spb[:, 0:t_b * 128],
                            in_=spT[:, 128 * ct:128 * (ct + t_b)])

                        # er1 select: er_ps[e, 4t:4t+4] = er1[dst(e)]
                        er_ps = erps.tile([128, 4 * TBMAX + 66 + TBMAX], F32, tag="er_ps")
                        for t in range(t_b):
                            nc.tensor.matmul(
                                out=er_ps[:, 4 * t:4 * t + 4],
                                lhsT=spb[:, 128 * t:128 * (t + 1)],
                                rhs=er_sb[:, 4 * b:4 * b + 4], start=True, stop=True)

                        # S build (edge-partition one-hot) for scatter
                        S_all = esb.tile([128, TBMAX * 128], BF16, tag="Sall3")
                        if SBUILD_TT:
                            nc.vector.tensor_tensor(
                                out=_ap3(S_all[:], 0, [128, t_b], [1, 128]),
                                in0=_ap3(iota_t[:], 0, [0, t_b], [1, 128]),
                                in1=_ap3(doff_t[:], ct, [1, t_b], [0, 128]),
                                op=mybir.AluOpType.is_equal)
                        else:
                            for t in range(t_b):
                                nc.vector.tensor_scalar(
                                    out=S_all[:, 128 * t:128 * (t + 1)], in0=iota_t[:],
                                    scalar1=doff_t[:, ct + t:ct + t + 1], scalar2=None,
                                    op0=mybir.AluOpType.is_equal)

                        if P3_STAGE < 3:
                            continue
                        # z = el_src + er_dst ; ex = exp(leaky_relu(z))
                        z = esb.tile([128, t_b * 4], F32, tag="z")
                        zr = z[:].rearrange("p (t h) -> p t h", h=4)
                        ELOFF = 128 if RW1 == 384 else 0
                        if a:
                            elA = _ap3(bufA[:].bitcast(F32), aoff * (RW1 // 2) + ELOFF, [RW1 // 2, a], [1, 4])
                            nc.vector.tensor_tensor(
                                out=zr[:, 0:a, :], in0=elA,
                                in1=_ap3(er_ps[:], 0, [4, a], [1, 4]),
                                op=mybir.AluOpType.add)
                        if bb:
                            elB = _ap3(bufB[:].bitcast(F32), boff * (RW1 // 2) + ELOFF, [RW1 // 2, bb], [1, 4])
                            nc.vector.tensor_tensor(
                                out=zr[:, a:t_b, :], in0=elB,
                                in1=_ap3(er_ps[:], 4 * a, [4, bb], [1, 4]),
                                op=mybir.AluOpType.add)
                        e1 = esb.tile([128, t_b * 4], F32, tag="e1")
                        e2 = esb.tile([128, t_b * 4], F32, tag="e2")
                        nc.scalar.activation(out=e1[:], in_=z[:], func=EXP)
                        nc.scalar.activation(out=e2[:], in_=z[:], func=EXP, scale=0.2)
                        ex = esb.tile([128, t_b * 4], BF16, tag="ex")
                        nc.vector.tensor_tensor(out=ex[:], in0=e1[:], in1=e2[:], op=mybir.AluOpType.max)

                        # rhs[e, 260t + 4d + h] = buf[e, t, (d,h)] * ex[e, t, h]; cols 256:260 = ex
                        rhs = esb.tile([128, t_b * 260], BF16, tag="rhs")
                        if a:
                            nc.vector.tensor_tensor(
                                out=bass.AP(rhs[:].tensor, rhs[:].offset,
                                            [rhs[:].ap[0], [260, a], [4, 64], [1, 4]]),
                                in0=bass.AP(bufA[:].tensor, bufA[:].offset + aoff * RW1,
                                            [bufA[:].ap[0], [RW1, a], [4, 64], [1, 4]]),
                                in1=bass.AP(ex[:].tensor, ex[:].offset,
                                            [ex[:].ap[0], [4, a], [0, 64], [1, 4]]),
                                op=mybir.AluOpType.mult)
                        if bb:
                            nc.vector.tensor_tensor(
                                out=bass.AP(rhs[:].tensor, rhs[:].offset + 260 * a,
                                            [rhs[:].ap[0], [260, bb], [4, 64], [1, 4]]),
                                in0=bass.AP(bufB[:].tensor, bufB[:].offset + boff * RW1,
                                            [bufB[:].ap[0], [RW1, bb], [4, 64], [1, 4]]),
                                in1=bass.AP(ex[:].tensor, ex[:].offset + 4 * a,
                                            [ex[:].ap[0], [4, bb], [0, 64], [1, 4]]),
                                op=mybir.AluOpType.mult)
                        nc.vector.tensor_copy(
                            rhs[:].rearrange("p (t c) -> p t c", c=260)[:, :, 256:260],
                            ex[:].rearrange("p (t h) -> p t h", h=4))

                        if P3_STAGE < 4:
                            continue
                        ps_o = eps.tile([128, 260], F32, tag="ps_o")
                        for t in range(t_b):
                            nc.tensor.matmul(
                                out=ps_o[:], lhsT=S_all[:, 128 * t:128 * (t + 1)],
                                rhs=rhs[:, 260 * t:260 * t + 260],
                                start=(t == 0), stop=(t == t_b - 1))

                        # normalize (cols are (d,h)-ordered; denominators at 256:260 by h)
                        splus = esb.tile([128, 4], F32, tag="splus")
                        nc.vector.tensor_scalar(
                            out=splus[:], in0=ps_o[:, 256:260], scalar1=1e-30,
                            scalar2=None, op0=mybir.AluOpType.add)
                        r = esb.tile([128, 4], F32, tag="r")
                        nc.vector.reciprocal(r[:], splus[:])
                        xn = esb.tile([128, 256], F32, tag="xn")
                        r_b = bass.AP(r[:].tensor, r[:].offset, [r[:].ap[0], [0, 64], [1, 4]])
                        nc.vector.tensor_tensor(
                            out=xn[:].rearrange("p (d h) -> p d h", h=4),
                            in0=ps_o[:, 0:256].rearrange("p (d h) -> p d h", h=4),
                            in1=r_b, op=mybir.AluOpType.mult)
                        if has_b1:
                            nc.vector.tensor_tensor(out=xn[:], in0=xn[:], in1=b1_t[:], op=mybir.AluOpType.add)
                        # elu(x) = exp(min(x,0)) + (max(x,0) - 1)
                        t1 = esb.tile([128, 256], F32, tag="t1")
                        nc.scalar.activation(out=t1[:], in_=xn[:], func=RELU, scale=-1.0)
                        u = esb.tile([128, 256], F32, tag="u")
                        nc.scalar.activation(out=u[:], in_=t1[:], func=EXP, scale=-1.0)
                        v = esb.tile([128, 256], F32, tag="v")
                        nc.vector.tensor_scalar(
                            out=v[:], in0=xn[:], scalar1=0.0, scalar2=-1.0,
                            op0=mybir.AluOpType.max, op1=mybir.AluOpType.add)
                        hb = esb.tile([128, 256], BF16, tag="hb")
                        nc.vector.tensor_tensor(out=hb[:], in0=u[:], in1=v[:], op=mybir.AluOpType.add)
                        if P3_STAGE < 5:
                            continue
                        pst = tps.tile([128, 256], BF16, tag="pst")
                        for k2 in range(2):
                            nc.tensor.transpose(out=pst[:, 128 * k2:128 * (k2 + 1)],
                                                in_=hb[:, 128 * k2:128 * (k2 + 1)], identity=ident[:])
                        for k2 in range(2):
                            nc.vector.tensor_copy(hT[k2][:, P * b:P * (b + 1)],
                                                  pst[:, 128 * k2:128 * (k2 + 1)])

                        # dense L2 (fused): ps2 cols 0:64 feat2, 64 el2, 65 er2
                        ps2 = er_ps[:, 4 * TBMAX:4 * TBMAX + 66]
                        for k in range(2):
                            nc.tensor.matmul(
                                out=ps2[:pb, :], lhsT=hT[k][:, P * b:P * b + pb],
                                rhs=rhsW2[k][:], start=(k == 0), stop=(k == 1))
                        er2b = esb.tile([128, 1], BF16, tag="er2b")
                        nc.vector.tensor_copy(er2b[:pb, :], ps2[:pb, 65:66])
                        # er2 select per tile (reuses S'); padded dst rows give garbage er2 — harmless
                        er2_ps = er_ps[:, 4 * TBMAX + 66:4 * TBMAX + 66 + TBMAX]
                        for t in range(t_b):
                            nc.tensor.matmul(
                                out=er2_ps[:, t:t + 1],
                                lhsT=spb[:, 128 * t:128 * (t + 1)],
                                rhs=er2b[:], start=True, stop=True)
                        nc.vector.tensor_copy(ere2[:, ct:ct + t_b], er2_ps[:, 0:t_b])

                        combo2 = combop2.tile([128, P2], BF16, tag="combo2")
                        nc.vector.tensor_copy(combo2[:pb, 0:64], ps2[:pb, 0:64])
                        nc.vector.tensor_copy(combo2[:pb, 64:66].bitcast(F32), ps2[:pb, 64:65])
                        if b < NBLK_A:
                            nc.sync.dma_start(out=T2_localA[P * b:P * b + pb, :], in_=combo2[:pb, :])
                        else:
                            r0 = P * b - HALF
                            nc.sync.dma_start(out=T2_localB[r0:r0 + pb, :], in_=combo2[:pb, :])
                        if b == NBLK_A - 1 and not SKIP_AG and not LATE_AG:
                            nc.gpsimd.collective_compute(
                                "AllGather", mybir.AluOpType.bypass,
                                replica_groups=[list(range(NCORES))],
                                ins=[T2_localA[:, :]], outs=[T2_fullA[:, :]])
                if not SKIP_AG and LATE_AG:
                    nc.gpsimd.collective_compute(
                        "AllGather", mybir.AluOpType.bypass,
                        replica_groups=[list(range(NCORES))],
                        ins=[T2_localA[:, :]], outs=[T2_fullA[:, :]])
                if not SKIP_AG and not SKIP_P3:
                    nc.gpsimd.collective_compute(
                        "AllGather", mybir.AluOpType.bypass,
                        replica_groups=[list(range(NCORES))],
                        ins=[T2_localB[:, :]], outs=[T2_fullB[:, :]])

            # ---------------- phase 6: L2 edge aggregation ----------------
            with (
                tc.tile_pool(name="g2buf", bufs=3) as g2buf,
                tc.tile_pool(name="e2sb", bufs=3) as e2sb,
                tc.tile_pool(name="e2ps", bufs=3, space="PSUM") as e2ps,
            ):
                for s0 in range(0, 0 if SKIP_P6 else NBLK, GB2):
                    sblocks = list(range(s0, min(s0 + GB2, NBLK)))
                    sa = int(sum(nA[b] for b in sblocks))
                    sb_ = int(sum(nB[b] for b in sblocks))
                    GCH = int(os.environ.get("K_GCH", "7"))
                    bufA = g2buf.tile([128, max(sa, 1) * RW2], BF16, tag="bufA2")
                    bufB = g2buf.tile([128, max(sb_, 1) * RW2], BF16, tag="bufB2")
                    for c0 in range(0, sa, GCH):
                        cn = min(GCH, sa - c0)
                        nc.gpsimd.dma_gather(
                            out_ap=bufA[:, RW2 * c0:RW2 * (c0 + cn)].rearrange("p (t e) -> p t e", e=RW2),
                            in_ap=T2_fullA[:, :],
                            idxs_ap=ilo_t[:, (clov[s0] + c0) * 8:(clov[s0] + c0 + cn) * 8],
                            num_idxs=cn * P, num_idxs_reg=cn * P, elem_size=RW2,
                            queue_num=nextq(), single_packet=SP)
                    for c0 in range(0, sb_, GCH):
                        cn = min(GCH, sb_ - c0)
                        nc.gpsimd.dma_gather(
                            out_ap=bufB[:, RW2 * c0:RW2 * (c0 + cn)].rearrange("p (t e) -> p t e", e=RW2),
                            in_ap=T2_fullB[:, :],
                            idxs_ap=ihi_t[:, (chiv[s0] + c0) * 8:(chiv[s0] + c0 + cn) * 8],
                            num_idxs=cn * P, num_idxs_reg=cn * P, elem_size=RW2,
                            queue_num=nextq(), single_packet=SP)

                    for b in sblocks:
                        if P6_STAGE < 1:
                            continue
                        a, bb = int(nA[b]), int(nB[b])
                        t_b = a + bb
                        ct = int(ctv[b])
                        pb = P if b < NBLK - 1 else ps_last
                        aoff = int(clov[b] - clov[s0])
                        boff = int(chiv[b] - chiv[s0])

                        S_all = e2sb.tile([128, TBMAX * 128], BF16, tag="Sall6")
                        if SBUILD_TT:
                            nc.vector.tensor_tensor(
                                out=_ap3(S_all[:], 0, [128, t_b], [1, 128]),
                                in0=_ap3(iota_t[:], 0, [0, t_b], [1, 128]),
                                in1=_ap3(doff_t[:], ct, [1, t_b], [0, 128]),
                                op=mybir.AluOpType.is_equal)
                        else:
                            for t in range(t_b):
                                nc.vector.tensor_scalar(
                                    out=S_all[:, 128 * t:128 * (t + 1)], in0=iota_t[:],
                                    scalar1=doff_t[:, ct + t:ct + t + 1], scalar2=None,
                                    op0=mybir.AluOpType.is_equal)

                        if P6_STAGE < 2:
                            continue
                        z = e2sb.tile([128, t_b], F32, tag="z2")
                        if a:
                            elA = _ap3(bufA[:].bitcast(F32), aoff * 64 + 32, [64, a], [1, 1])
                            nc.vector.tensor_tensor(
                                out=z[:, 0:a].rearrange("p (t h) -> p t h", h=1),
                                in0=elA,
                                in1=_ap3(ere2[:], ct, [1, a], [1, 1]),
                                op=mybir.AluOpType.add)
                        if bb:
                            elB = _ap3(bufB[:].bitcast(F32), boff * 64 + 32, [64, bb], [1, 1])
                            nc.vector.tensor_tensor(
                                out=z[:, a:t_b].rearrange("p (t h) -> p t h", h=1),
                                in0=elB,
                                in1=_ap3(ere2[:], ct + a, [1, bb], [1, 1]),
                                op=mybir.AluOpType.add)
                        e1 = e2sb.tile([128, t_b], F32, tag="e21")
                        e2 = e2sb.tile([128, t_b], F32, tag="e22")
                        nc.scalar.activation(out=e1[:], in_=z[:], func=EXP)
                        nc.scalar.activation(out=e2[:], in_=z[:], func=EXP, scale=0.2)
                        m = e2sb.tile([128, t_b], F32, tag="m2")
                        nc.vector.tensor_tensor(out=m[:], in0=e1[:], in1=e2[:], op=mybir.AluOpType.max)
                        # ex64[e, t, d] = ex[e, t] broadcast over d
                        ex64 = e2sb.tile([128, t_b * 64], BF16, tag="ex64")
                        nc.scalar.activation(
                            out=ex64[:].rearrange("p (t d) -> p t d", d=64),
                            in_=bass.AP(m[:].tensor, m[:].offset, [m[:].ap[0], [1, t_b], [0, 64]]),
                            func=COPY)
                        rhs = e2sb.tile([128, t_b * 65], BF16, tag="rhs2")
                        if a:
                            nc.vector.tensor_tensor(
                                out=_ap3(rhs[:], 0, [65, a], [1, 64]),
                                in0=bass.AP(bufA[:].tensor, bufA[:].offset + aoff * RW2,
                                            [bufA[:].ap[0], [RW2, a], [1, 64]]),
                                in1=_ap3(ex64[:], 0, [64, a], [1, 64]),
                                op=mybir.AluOpType.mult)
                        if bb:
                            nc.vector.tensor_tensor(
                                out=_ap3(rhs[:], 65 * a, [65, bb], [1, 64]),
                                in0=bass.AP(bufB[:].tensor, bufB[:].offset + boff * RW2,
                                            [bufB[:].ap[0], [RW2, bb], [1, 64]]),
                                in1=_ap3(ex64[:], 64 * a, [64, bb], [1, 64]),
                                op=mybir.AluOpType.mult)
                        nc.vector.tensor_copy(
                            rhs[:].rearrange("p (t c) -> p t c", c=65)[:, :, 64:65],
                            ex64[:].rearrange("p (t d) -> p t d", d=64)[:, :, 0:1])

                        if P6_STAGE < 3:
                            continue
                        ps_o = e2ps.tile([128, 65], F32, tag="ps_o2")
                        for t in range(t_b):
                            nc.tensor.matmul(
                                out=ps_o[:], lhsT=S_all[:, 128 * t:128 * (t + 1)],
                                rhs=rhs[:, 65 * t:65 * t + 65],
                                start=(t == 0), stop=(t == t_b - 1))

                        splus = e2sb.tile([128, 1], F32, tag="splus2")
                        nc.vector.tensor_scalar(
                            out=splus[:], in0=ps_o[:, 64:65], scalar1=1e-30,
                            scalar2=None, op0=mybir.AluOpType.add)
                        r = e2sb.tile([128, 1], F32, tag="r2")
                        nc.vector.reciprocal(r[:], splus[:])
                        outf = e2sb.tile([128, 64], F32, tag="outf")
                        nc.vector.tensor_scalar(
                            out=outf[:], in0=ps_o[:, 0:64], scalar1=r[:, 0:1],
                            scalar2=None, op0=mybir.AluOpType.mult)
                        if has_b2:
                            nc.vector.tensor_tensor(out=outf[:], in0=outf[:], in1=b2_t[:], op=mybir.AluOpType.add)
                        nc.sync.dma_start(out=out_t[P * b:P * b + pb, :], in_=outf[:pb, :])

    nc.compile()
    return nc


def _make_in_maps(np_inputs, nA, nB, NT, idx_lo, idx_hi, doff, sprime,
                  has_b1, has_b2):
    x = np.asarray(np_inputs["x"], dtype=np.float32)
    W1 = np.asarray(np_inputs["W1"], dtype=np.float32)
    al1 = np.asarray(np_inputs["al1"], dtype=np.float32)
    ar1 = np.asarray(np_inputs["ar1"], dtype=np.float32)
    b1 = np.asarray(np_inputs["b1"], dtype=np.float32)
    W2 = np.asarray(np_inputs["W2"], dtype=np.float32)
    al2 = np.asarray(np_inputs["al2"], dtype=np.float32)
    ar2 = np.asarray(np_inputs["ar2"], dtype=np.float32)
    b2 = np.asarray(np_inputs["b2"], dtype=np.float32)

    p = _perm_dh()
    w1ext = np.zeros((IN, 264), np.float32)
    w1ext[:, 0:256] = W1[:, p]
    for h in range(H1):
        w1ext[:, 256 + h] = W1[:, 64 * h:64 * h + 64] @ al1[h]
        w1ext[:, 260 + h] = W1[:, 64 * h:64 * h + 64] @ ar1[h]
    # L2: el2/er2 contract over the (permuted) 256-dim input of layer 2
    # reference: feat2 = h @ W2 [N, 64]; el2 = (feat2 * al2).sum(-1) = h @ (W2 @ al2)
    w2ext = np.zeros((HD1, 66), np.float32)
    w2ext[:, 0:64] = W2[p, :]
    w2ext[:, 64] = (W2 @ al2[0])[p]
    w2ext[:, 65] = (W2 @ ar2[0])[p]

    xt_np = np.ascontiguousarray(x.T)
    in_maps = []
    for d in range(NCORES):
        m = {
            "xT": np.ascontiguousarray(xt_np[:, NSH * d:NSH * (d + 1)]),
            "w1e": w1ext, "w2e": w2ext,
            "ilo": np.ascontiguousarray(idx_lo[d]),
            "ihi": np.ascontiguousarray(idx_hi[d]),
            "idoff": np.ascontiguousarray(doff[d]),
            "spT": np.ascontiguousarray(sprime[d]),
        }
        if has_b1:
            m["b1r"] = np.tile(b1.reshape(1, HD1)[:, p], (128, 1)).astype(np.float32)
        if has_b2:
            m["b2r"] = np.tile(b2.reshape(1, D2), (128, 1)).astype(np.float32)
        in_maps.append(m)
    return in_maps


def kernel(x, src, dst, W1, al1, ar1, b1, W2, al2, ar2, b2):
    np_inputs = {"x": x, "src": src, "dst": dst, "W1": W1, "al1": al1, "ar1": ar1,
                 "b1": b1, "W2": W2, "al2": al2, "ar2": ar2, "b2": b2}
    nA, nB, NT, idx_lo, idx_hi, doff, sprime = _prep_edges(src, dst)
    CL, CH = idx_lo[0].shape[1], idx_hi[0].shape[1]
    has_b1 = bool(np.any(np.asarray(b1)))
    has_b2 = bool(np.any(np.asarray(b2)))

    nc = _build(nA, nB, NT, CL, CH, has_b1, has_b2)
    in_maps = _make_in_maps(np_inputs, nA, nB, NT, idx_lo, idx_hi, doff, sprime,
                            has_b1, has_b2)

    trace = bool(int(os.environ.get("K_TRACE", "0")))
    res = run_bass_kernel_spmd(
        nc, in_maps, core_ids=list(range(NCORES)), trace=trace,
        trace_cores=list(range(NCORES)) if trace else None, stitch_traces=trace)
    out = np.concatenate([res.results[d]["out"] for d in range(NCORES)], axis=0)
    return out
